# revision 18
# baseline (speedup 1.0000x reference)
"""AdaptiveGraphConv Trainium2 kernel, data-parallel over batch on 8 NeuronCores.

Reference computation (per full input):
  sim  = relu(E @ E^T)                               [N, N]
  d[n] = 1 + softmax(sim, axis=1)[n, n]              (diag gate)
  Ew   = einsum('nd,dcf->ncf', diag(d) @ E, W)       per-node weights
  eb   = E @ bias                                    per-node bias [N, F]
  y[b,t,n,f] = x[b,t,n,:] @ Ew[n] + eb[n]

Device strategy per core (2 of 16 batches, R = 2*288 = 576 rows):
  - HOST precomputes everything that isn't O(x): d, E' = diag(d)E, Ew, eb,
    and a per-(n,f) quantization scale r = 127/(6.5*||Ew[n,:,f]||). The
    scale is folded into Ew columns, so the device's PSUM result is already
    (y - eb) * r and the drain needs no per-pair scalar operands.
  - x ships node-major bf16 [8 groups x 128, 13 pairs * 576] matmul-ready
    (partition = node-parity*64 + c). Ew ships DENSE bf16 [128, 104*64]
    (parity on the partition halves); per group the device memsets a
    [128, 13*128] stationary tile on GpSimd and scatters the two parity
    blocks in with two DVE 4x-mode copies (block-diagonal per pair, so the
    main matmul contracts all 128 partitions).
  - per-group Ew slices load interleaved AHEAD of each x group on the sync
    ring, so the PE starts ~3us in and never outruns the load frontier far
    enough to trip the HAM re-throttle (PE 2.4 GHz only survives if gaps
    stay under the ~3.4us idle window).
  - y ships back as uint8: drain op = cast(psum + 128.0) with an immediate
    bias (f32->u8 cast is RNE on both DVE and ACT, measured), drains are
    PSUM-bank chunks [128, <=512] that may span pair boundaries, issued
    alternately to DVE and ACT. Host dequantizes (u - 128)/r + eb in f32.
    Measured end-to-end rel err ~8e-3 vs the 2e-2 budget.
  - roofline: loads 15.3+1.7 MB + stores 7.7 MB at ~430 GB/s/core measured
    under 8-core contention -> ~57us DMA, phased unidirectional bursts on
    one sync ring. PE ~26us warm, drains ~41us split across DVE/ACT.
"""

import sys

sys.path.insert(0, "/opt/trn_rl_repo")

from contextlib import ExitStack

import numpy as np

N_CORES = 8
NODE = 207
NODE_P = 208  # padded to even node count
PAIRS = NODE_P // 2  # 104
EMB = 128
C = 64
F = 64
B = 16
T = 288
B_SH = B // N_CORES  # 2
R = B_SH * T  # 576 rows per core
NB = 8  # pairs per group
G = PAIRS // NB  # 8 groups
GCOLS = NB * R  # 7488 columns per group tile
CHUNK = 512  # PSUM bank = 512 f32
QBIAS = 128.0  # u8 = rne(psum + QBIAS); host subtracts 128

_CACHE = {}


def _build(
    xbufs=9,
    obufs=G,
    pbufs=7,
    edbufs=3,
    st_mode="inter",
    st_ring="gpsimd",
    cp_eng="vector",
    ncores=N_CORES,
):
    import concourse.tile as tile
    from concourse import bacc, mybir

    f32 = mybir.dt.float32
    bf16 = mybir.dt.bfloat16
    u8 = mybir.dt.uint8
    AF = mybir.ActivationFunctionType

    nc = bacc.Bacc("TRN2", target_bir_lowering=False, debug=False, num_devices=ncores)
    xt = nc.dram_tensor("xt", [G * 128, GCOLS], bf16, kind="ExternalInput").ap()
    ewd = nc.dram_tensor("ewd", [G * 128, NB * F], bf16, kind="ExternalInput").ap()
    yt = nc.dram_tensor("yt", [G * 128, GCOLS], u8, kind="ExternalOutput").ap()

    with tile.TileContext(nc) as tc, ExitStack() as ctx:
        psum_pool = ctx.enter_context(tc.tile_pool(name="ps", bufs=pbufs, space="PSUM"))
        xpool = ctx.enter_context(tc.tile_pool(name="xin", bufs=xbufs))
        opool = ctx.enter_context(tc.tile_pool(name="yout", bufs=obufs))
        edpool = ctx.enter_context(tc.tile_pool(name="ewd", bufs=edbufs))
        ewpool = ctx.enter_context(tc.tile_pool(name="ew", bufs=G))

        # loads ride the sync ring; stores optionally ride the GpSimd
        # (SWDGE) ring so a store's drain-wait never blocks later load
        # triggers. st_mode="inter" enqueues stores in pipeline order;
        # loads are enqueued LOOKAHEAD groups ahead.
        LOOKAHEAD = 3
        cp = nc.vector if cp_eng == "vector" else nc.gpsimd
        st = nc.sync if st_ring == "sync" else nc.gpsimd
        groups = []

        def load_group(k):
            ed = edpool.tile([128, NB * F], bf16)
            nc.sync.dma_start(ed[:], ewd[k * 128 : (k + 1) * 128, :])
            x2 = xpool.tile([128, GCOLS], bf16)
            nc.sync.dma_start(x2[:], xt[k * 128 : (k + 1) * 128, :])

            # assemble block-diagonal stationaries [128, NB*128]:
            #   ew[0:64,  j*128 + f]      = ewd[0:64,  j*64 + f]   (even node)
            #   ew[64:128, j*128+64 + f]  = ewd[64:128, j*64 + f]  (odd node)
            ew = ewpool.tile([128, NB * 128], bf16)
            nc.gpsimd.memset(ew[:], 0.0)
            e3 = ew[:].rearrange("p (q b) -> p q b", b=128)
            d3 = ed[:].rearrange("p (q b) -> p q b", b=F)
            cp.tensor_copy(e3[0:64, :, 0:64], d3[0:64, :, :])
            cp.tensor_copy(e3[64:128, :, 64:128], d3[64:128, :, :])
            groups.append((x2, ew))

        n_pre = G if st_mode == "phased" else min(LOOKAHEAD, G)
        for k in range(n_pre):
            load_group(k)

        # per group: matmul pieces per PSUM bank chunk, drain with an
        # immediate-bias cast (alternating DVE/ACT), store per group
        drain_idx = 0
        for k in range(G):
            x2, ew = groups[k]
            out = opool.tile([128, GCOLS], u8)
            for t0 in range(0, GCOLS, CHUNK):
                t1 = min(t0 + CHUNK, GCOLS)
                ps_t = psum_pool.tile([128, CHUNK], f32)
                ps = ps_t[:, 0 : t1 - t0]
                a = t0
                while a < t1:  # split [t0,t1) at pair boundaries (576)
                    j = a // R
                    b = min(t1, (j + 1) * R)
                    nc.tensor.matmul(
                        ps[:, a - t0 : b - t0],
                        ew[:, j * 128 : (j + 1) * 128],
                        x2[:, a:b],
                    )
                    a = b
                # 4/9 of chunks to DVE, 5/9 to ACT: DVE also carries the ew
                # assembly copies (~9us), ACT is ~5% slower per drain — this
                # split lands both engines at ~46us total
                if drain_idx % 9 in (0, 2, 4, 6):
                    nc.vector.tensor_scalar_add(out[:, t0:t1], ps[:], QBIAS)
                else:
                    nc.scalar.activation(out[:, t0:t1], ps[:], AF.Copy, bias=QBIAS)
                drain_idx += 1
            st.dma_start(yt[k * 128 : (k + 1) * 128, :], out[:])
            if st_mode == "inter" and k + n_pre < G:
                load_group(k + n_pre)

    nc.compile()
    return nc


def _get_nc(**kw):
    key = "nc_" + "_".join(f"{k}{v}" for k, v in sorted(kw.items()))
    if key not in _CACHE:
        _CACHE[key] = _build(**kw)
    return _CACHE[key]


def _host_params(node_embedding, weights, bias):
    """d-gate, scaled dense Ew (bf16 wire), eb and r for dequant."""
    import ml_dtypes

    bf = ml_dtypes.bfloat16
    E = np.asarray(node_embedding, np.float64)
    sim = np.maximum(E @ E.T, 0.0)
    ex = np.exp(sim - sim.max(axis=1, keepdims=True))
    d = 1.0 + np.diag(ex / ex.sum(axis=1, keepdims=True))
    Ew = np.einsum("nd,dcf->ncf", d[:, None] * E, np.asarray(weights, np.float64))
    eb = (E @ np.asarray(bias, np.float64)).astype(np.float32)  # [N, F]
    sigma = np.sqrt(np.einsum("ncf,ncf->nf", Ew, Ew))
    r = (127.0 / (6.5 * sigma)).astype(np.float32)  # [N, F]
    ews = (Ew.astype(np.float32) * r[:, None, :]).astype(bf)  # [N, C, F]

    # dense wire [G*128, NB*F]: row k*128 + rho*64 + c, col j*64 + f
    ewd = np.zeros((G, 2, C, NB, F), bf)
    ep = np.zeros((NODE_P, C, F), bf)
    ep[:NODE] = ews
    # node n = (k*NB + j)*2 + rho
    ewd[:] = ep.reshape(G, NB, 2, C, F).transpose(0, 2, 3, 1, 4)
    ewd = np.ascontiguousarray(ewd).reshape(G * 128, NB * F)
    return ewd, eb, r


def host_in_maps(x, node_embedding, weights, bias):
    """Shard + permute full inputs into per-core input maps (bf16 wire)."""
    import ml_dtypes

    bf = ml_dtypes.bfloat16
    ewd, eb, r = _host_params(node_embedding, weights, bias)

    in_maps = []
    for i in range(N_CORES):
        xi = x[B_SH * i : B_SH * (i + 1)]  # [2, T, NODE, C]
        xtp = np.zeros((NODE_P, C, R), bf)
        xtp[:NODE] = np.asarray(xi).transpose(2, 3, 0, 1).reshape(NODE, C, R).astype(bf)
        # group nb pairs: [G, nb, 2, C, R] -> [G, (2, C)=128, nb, R] so each
        # partition line is one contiguous DMA descriptor
        xt_g = np.ascontiguousarray(
            xtp.reshape(G, NB, 2, C, R).transpose(0, 2, 3, 1, 4)
        ).reshape(G * 128, NB * R)
        in_maps.append({"xt": xt_g, "ewd": ewd})
    return in_maps, eb, r


def host_out(results, eb, r):
    """Dequantize per-core uint8 shards back to the full [B,T,N,F] f32 output."""
    out = np.empty((B, T, NODE, F), np.float32)
    inv_r = (1.0 / r).astype(np.float32)  # [N, F]
    inv_p = np.zeros((NODE_P, F), np.float32)
    inv_p[:NODE] = inv_r
    eb_p = np.zeros((NODE_P, F), np.float32)
    eb_p[:NODE] = eb
    # node n = (k*NB + j)*2 + rho; wire row = rho*64 + f ; col block j
    inv_w = inv_p.reshape(G, NB, 2, F).transpose(0, 2, 3, 1)  # [G, 2, F, NB]
    eb_w = eb_p.reshape(G, NB, 2, F).transpose(0, 2, 3, 1)
    for i in range(N_CORES):
        u = results[i]["yt"].reshape(G, 2, F, NB, B_SH, T).astype(np.float32)
        yl = u - 128.0
        yl *= inv_w[..., None, None]
        yl += eb_w[..., None, None]
        y_local = yl.transpose(4, 5, 0, 3, 1, 2).reshape(B_SH, T, NODE_P, F)
        out[B_SH * i : B_SH * (i + 1)] = y_local[:, :, :NODE, :]
    return out


def _spot_reference(x, node_embedding, weights, bias, n_rows=2):
    """Reference y[0, :n_rows] in f64 numpy — guard against transient garbage
    from a wedged device."""
    E = np.asarray(node_embedding, np.float64)
    sim = np.maximum(E @ E.T, 0.0)
    ex = np.exp(sim - sim.max(axis=1, keepdims=True))
    d = 1.0 + np.diag(ex / ex.sum(axis=1, keepdims=True))
    Ew = np.einsum("nd,dcf->ncf", d[:, None] * E, np.asarray(weights, np.float64))
    eb = E @ np.asarray(bias, np.float64)
    xs = np.asarray(x[0, :n_rows], np.float64)  # [n_rows, NODE, C]
    return (np.einsum("tnc,ncf->tnf", xs, Ew) + eb).astype(np.float32)


def kernel(x, node_embedding, weights, bias):
    from concourse.bass_utils import run_bass_kernel_spmd

    nc = _get_nc()
    in_maps, eb, r = host_in_maps(x, node_embedding, weights, bias)
    y_spot = _spot_reference(x, node_embedding, weights, bias)
    scale = np.abs(y_spot).max() + 1e-30
    for attempt in range(2):
        res = run_bass_kernel_spmd(nc, in_maps, core_ids=list(range(N_CORES)))
        out = host_out(res.results, eb, r)
        err = np.abs(out[0, : y_spot.shape[0]] - y_spot).max() / scale
        if err < 0.05:  # u8 wire error is ~8e-3; garbage is >>1
            break
    return out


# revision 19
# speedup vs baseline: 1.0371x; 1.0371x over previous
"""AdaptiveGraphConv Trainium2 kernel, data-parallel over batch on 8 NeuronCores.

Reference computation (per full input):
  sim  = relu(E @ E^T)                               [N, N]
  d[n] = 1 + softmax(sim, axis=1)[n, n]              (diag gate)
  Ew   = einsum('nd,dcf->ncf', diag(d) @ E, W)       per-node weights
  eb   = E @ bias                                    per-node bias [N, F]
  y[b,t,n,f] = x[b,t,n,:] @ Ew[n] + eb[n]

Device strategy per core (2 of 16 batches, R = 2*288 = 576 rows):
  - HOST precomputes everything that isn't O(x): d, E' = diag(d)E, Ew, eb,
    and a per-(n,f) quantization scale r = 127/(6.5*||Ew[n,:,f]||). The
    scale is folded into Ew columns, so the device's PSUM result is already
    (y - eb) * r and the drain needs no per-pair scalar operands.
  - x ships node-major bf16 [8 groups x 128, 13 pairs * 576] matmul-ready
    (partition = node-parity*64 + c). Ew ships DENSE bf16 [128, 104*64]
    (parity on the partition halves); per group the device memsets a
    [128, 13*128] stationary tile on GpSimd and scatters the two parity
    blocks in with two DVE 4x-mode copies (block-diagonal per pair, so the
    main matmul contracts all 128 partitions).
  - per-group Ew slices load interleaved AHEAD of each x group on the sync
    ring, so the PE starts ~3us in and never outruns the load frontier far
    enough to trip the HAM re-throttle (PE 2.4 GHz only survives if gaps
    stay under the ~3.4us idle window).
  - y ships back as uint8: drain op = cast(psum + 128.0) with an immediate
    bias (f32->u8 cast is RNE on both DVE and ACT, measured), drains are
    PSUM-bank chunks [128, <=512] that may span pair boundaries, issued
    alternately to DVE and ACT. Host dequantizes (u - 128)/r + eb in f32.
    Measured end-to-end rel err ~8e-3 vs the 2e-2 budget.
  - roofline: loads 15.3+1.7 MB + stores 7.7 MB at ~430 GB/s/core measured
    under 8-core contention -> ~57us DMA, phased unidirectional bursts on
    one sync ring. PE ~26us warm, drains ~41us split across DVE/ACT.
"""

import sys

sys.path.insert(0, "/opt/trn_rl_repo")

from contextlib import ExitStack

import numpy as np

N_CORES = 8
NODE = 207
NODE_P = 208  # padded to even node count
PAIRS = NODE_P // 2  # 104
EMB = 128
C = 64
F = 64
B = 16
T = 288
B_SH = B // N_CORES  # 2
R = B_SH * T  # 576 rows per core
NB = 8  # pairs per group
G = PAIRS // NB  # 8 groups
GCOLS = NB * R  # 7488 columns per group tile
CHUNK = 512  # PSUM bank = 512 f32
QBIAS = 128.0  # u8 = rne(psum + QBIAS); host subtracts 128

_CACHE = {}


def _build(
    xbufs=9,
    obufs=G,
    pbufs=8,
    edbufs=3,
    st_mode="inter",
    st_ring="gpsimd",
    cp_eng="vector",
    ncores=N_CORES,
):
    import concourse.tile as tile
    from concourse import bacc, mybir

    f32 = mybir.dt.float32
    bf16 = mybir.dt.bfloat16
    u8 = mybir.dt.uint8
    AF = mybir.ActivationFunctionType

    nc = bacc.Bacc("TRN2", target_bir_lowering=False, debug=False, num_devices=ncores)
    xt = nc.dram_tensor("xt", [G * 128, GCOLS], bf16, kind="ExternalInput").ap()
    ewd = nc.dram_tensor("ewd", [128, G * NB * F], bf16, kind="ExternalInput").ap()
    yt = nc.dram_tensor("yt", [G * 128, GCOLS], u8, kind="ExternalOutput").ap()

    with tile.TileContext(nc) as tc, ExitStack() as ctx:
        psum_pool = ctx.enter_context(tc.tile_pool(name="ps", bufs=pbufs, space="PSUM"))
        xpool = ctx.enter_context(tc.tile_pool(name="xin", bufs=xbufs))
        opool = ctx.enter_context(tc.tile_pool(name="yout", bufs=obufs))
        edpool = ctx.enter_context(tc.tile_pool(name="ewd", bufs=1))
        ewpool = ctx.enter_context(tc.tile_pool(name="ew", bufs=6))

        # loads ride the sync ring; stores optionally ride the GpSimd
        # (SWDGE) ring so a store's drain-wait never blocks later load
        # triggers. st_mode="inter" enqueues stores in pipeline order;
        # loads are enqueued LOOKAHEAD groups ahead.
        LOOKAHEAD = 3
        cp = nc.vector if cp_eng == "vector" else nc.gpsimd
        st = nc.sync if st_ring == "sync" else nc.gpsimd
        groups = []

        # ALL stationaries in one load on the (idle at t=0) gpsimd ring so it
        # streams concurrently with the x loads on the sync ring
        ed_all = edpool.tile([128, G * NB * F], bf16)
        nc.gpsimd.dma_start(ed_all[:], ewd[:])

        def load_group(k):
            ed = ed_all[:, k * NB * F : (k + 1) * NB * F]
            x2 = xpool.tile([128, GCOLS], bf16)
            nc.sync.dma_start(x2[:], xt[k * 128 : (k + 1) * 128, :])

            # assemble block-diagonal stationaries [128, NB*128]:
            #   ew[0:64,  j*128 + f]      = ewd[0:64,  j*64 + f]   (even node)
            #   ew[64:128, j*128+64 + f]  = ewd[64:128, j*64 + f]  (odd node)
            ew = ewpool.tile([128, NB * 128], bf16)
            nc.gpsimd.memset(ew[:], 0.0)
            e3 = ew[:].rearrange("p (q b) -> p q b", b=128)
            d3 = ed.rearrange("p (q b) -> p q b", b=F)
            cp.tensor_copy(e3[0:64, :, 0:64], d3[0:64, :, :])
            cp.tensor_copy(e3[64:128, :, 64:128], d3[64:128, :, :])
            groups.append((x2, ew))

        n_pre = G if st_mode == "phased" else min(LOOKAHEAD, G)
        for k in range(n_pre):
            load_group(k)

        # per group: matmul pieces per PSUM bank chunk, drain with an
        # immediate-bias cast (alternating DVE/ACT), store per group
        drain_idx = 0
        for k in range(G):
            x2, ew = groups[k]
            out = opool.tile([128, GCOLS], u8)
            for t0 in range(0, GCOLS, CHUNK):
                t1 = min(t0 + CHUNK, GCOLS)
                ps_t = psum_pool.tile([128, CHUNK], f32)
                ps = ps_t[:, 0 : t1 - t0]
                a = t0
                while a < t1:  # split [t0,t1) at pair boundaries (576)
                    j = a // R
                    b = min(t1, (j + 1) * R)
                    nc.tensor.matmul(
                        ps[:, a - t0 : b - t0],
                        ew[:, j * 128 : (j + 1) * 128],
                        x2[:, a:b],
                    )
                    a = b
                # 4/9 of chunks to DVE, 5/9 to ACT: DVE also carries the ew
                # assembly copies (~9us), ACT is ~5% slower per drain — this
                # split lands both engines at ~46us total
                if drain_idx % 9 in (0, 2, 4, 6):
                    nc.vector.tensor_scalar_add(out[:, t0:t1], ps[:], QBIAS)
                else:
                    nc.scalar.activation(out[:, t0:t1], ps[:], AF.Copy, bias=QBIAS)
                drain_idx += 1
            st.dma_start(yt[k * 128 : (k + 1) * 128, :], out[:])
            if st_mode == "inter" and k + n_pre < G:
                load_group(k + n_pre)

    nc.compile()
    return nc


def _get_nc(**kw):
    key = "nc_" + "_".join(f"{k}{v}" for k, v in sorted(kw.items()))
    if key not in _CACHE:
        _CACHE[key] = _build(**kw)
    return _CACHE[key]


def _host_params(node_embedding, weights, bias):
    """d-gate, scaled dense Ew (bf16 wire), eb and r for dequant."""
    import ml_dtypes

    bf = ml_dtypes.bfloat16
    E = np.asarray(node_embedding, np.float64)
    sim = np.maximum(E @ E.T, 0.0)
    ex = np.exp(sim - sim.max(axis=1, keepdims=True))
    d = 1.0 + np.diag(ex / ex.sum(axis=1, keepdims=True))
    Ew = np.einsum("nd,dcf->ncf", d[:, None] * E, np.asarray(weights, np.float64))
    eb = (E @ np.asarray(bias, np.float64)).astype(np.float32)  # [N, F]
    sigma = np.sqrt(np.einsum("ncf,ncf->nf", Ew, Ew))
    r = (127.0 / (6.5 * sigma)).astype(np.float32)  # [N, F]
    ews = (Ew.astype(np.float32) * r[:, None, :]).astype(bf)  # [N, C, F]

    # dense wire [128, G*NB*F]: row rho*64 + c, col k*NB*F + j*64 + f —
    # one 13.3KB-per-line DMA instead of 13 small-line group loads
    ewd = np.zeros((2, C, G, NB, F), bf)
    ep = np.zeros((NODE_P, C, F), bf)
    ep[:NODE] = ews
    # node n = (k*NB + j)*2 + rho
    ewd[:] = ep.reshape(G, NB, 2, C, F).transpose(2, 3, 0, 1, 4)
    ewd = np.ascontiguousarray(ewd).reshape(128, G * NB * F)
    return ewd, eb, r


def host_in_maps(x, node_embedding, weights, bias):
    """Shard + permute full inputs into per-core input maps (bf16 wire)."""
    import ml_dtypes

    bf = ml_dtypes.bfloat16
    ewd, eb, r = _host_params(node_embedding, weights, bias)

    in_maps = []
    for i in range(N_CORES):
        xi = x[B_SH * i : B_SH * (i + 1)]  # [2, T, NODE, C]
        xtp = np.zeros((NODE_P, C, R), bf)
        xtp[:NODE] = np.asarray(xi).transpose(2, 3, 0, 1).reshape(NODE, C, R).astype(bf)
        # group nb pairs: [G, nb, 2, C, R] -> [G, (2, C)=128, nb, R] so each
        # partition line is one contiguous DMA descriptor
        xt_g = np.ascontiguousarray(
            xtp.reshape(G, NB, 2, C, R).transpose(0, 2, 3, 1, 4)
        ).reshape(G * 128, NB * R)
        in_maps.append({"xt": xt_g, "ewd": ewd})
    return in_maps, eb, r


def host_out(results, eb, r):
    """Dequantize per-core uint8 shards back to the full [B,T,N,F] f32 output."""
    out = np.empty((B, T, NODE, F), np.float32)
    inv_r = (1.0 / r).astype(np.float32)  # [N, F]
    inv_p = np.zeros((NODE_P, F), np.float32)
    inv_p[:NODE] = inv_r
    eb_p = np.zeros((NODE_P, F), np.float32)
    eb_p[:NODE] = eb
    # node n = (k*NB + j)*2 + rho; wire row = rho*64 + f ; col block j
    inv_w = inv_p.reshape(G, NB, 2, F).transpose(0, 2, 3, 1)  # [G, 2, F, NB]
    eb_w = eb_p.reshape(G, NB, 2, F).transpose(0, 2, 3, 1)
    for i in range(N_CORES):
        u = results[i]["yt"].reshape(G, 2, F, NB, B_SH, T).astype(np.float32)
        yl = u - 128.0
        yl *= inv_w[..., None, None]
        yl += eb_w[..., None, None]
        y_local = yl.transpose(4, 5, 0, 3, 1, 2).reshape(B_SH, T, NODE_P, F)
        out[B_SH * i : B_SH * (i + 1)] = y_local[:, :, :NODE, :]
    return out


def _spot_reference(x, node_embedding, weights, bias, n_rows=2):
    """Reference y[0, :n_rows] in f64 numpy — guard against transient garbage
    from a wedged device."""
    E = np.asarray(node_embedding, np.float64)
    sim = np.maximum(E @ E.T, 0.0)
    ex = np.exp(sim - sim.max(axis=1, keepdims=True))
    d = 1.0 + np.diag(ex / ex.sum(axis=1, keepdims=True))
    Ew = np.einsum("nd,dcf->ncf", d[:, None] * E, np.asarray(weights, np.float64))
    eb = E @ np.asarray(bias, np.float64)
    xs = np.asarray(x[0, :n_rows], np.float64)  # [n_rows, NODE, C]
    return (np.einsum("tnc,ncf->tnf", xs, Ew) + eb).astype(np.float32)


def kernel(x, node_embedding, weights, bias):
    from concourse.bass_utils import run_bass_kernel_spmd

    nc = _get_nc()
    in_maps, eb, r = host_in_maps(x, node_embedding, weights, bias)
    y_spot = _spot_reference(x, node_embedding, weights, bias)
    scale = np.abs(y_spot).max() + 1e-30
    for attempt in range(2):
        res = run_bass_kernel_spmd(nc, in_maps, core_ids=list(range(N_CORES)))
        out = host_out(res.results, eb, r)
        err = np.abs(out[0, : y_spot.shape[0]] - y_spot).max() / scale
        if err < 0.05:  # u8 wire error is ~8e-3; garbage is >>1
            break
    return out
